# revision 1
# baseline (speedup 1.0000x reference)
"""Trainium2 Bass kernel for NT-Xent style contrastive loss.

Math (B=4096, D=128, T=0.25):
  z_i = normalize(emb_i), z_j = normalize(emb_j)   (L2, per row)
  S = z_i @ z_j^T                                   [B, B]
  loss = (1/2B) * sum_r [ -2*S[r,r]/T + log(rowsum_c exp(S[r,c]/T))
                                      + log(rowsum_c exp(S^T[r,c]/T)) ]

Sharding: data-parallel over rows. Each of the 8 cores receives the full
(emb_i, emb_j) rotated so that "its" 512 rows sit at rows 0:512, computes
both exp-rowsum branches for those rows against all 4096 columns, the diag
(positives) terms, and writes a per-partition partial [128,1]. The host sums
the 8x128 partials and divides by 2B (the trivial all-reduce/gather step).

Per-core dataflow:
  - SWDGE cast-DMA loads emb (fp32 DRAM -> bf16 SBUF, row layout [128,32,128])
  - DVE: square + reduce -> row norms^2 [128,32]
  - ACT: inv_norm = Exp(-0.5 * Ln(n2))  (stays in the exp/ln table set)
  - DVE: z = emb * inv_norm (per-partition scalar, 32 tiles)
  - DMA xbar transpose (bf16): zT [d=128, 4096 rows]; a tiny dummy transpose
    reading the last-loaded chunk absorbs the xbar copy->transpose transition
    wait (XPOSE instructions only have one sync-wait slot)
  - PE: S chunks [128, 2048] = zT_i[:,m,:].T @ zT_j slice, fp32 PSUM
  - ACT: Exp(scale=4.0) over PSUM chunk with accum_out -> fused row sums
  - tail: Ln(rowsums), diag via DVE mul+reduce, combine, DMA out [128,1]
"""

import numpy as np

B = 4096
D = 128
P = 128
NCORES = 8
RPC = B // NCORES          # 512 rows per core
NT = B // P                # 32 row-tiles
MT = RPC // P              # 4 m-tiles owned per core
TEMP = 0.25
INV_T = 1.0 / TEMP         # 4.0

_cache = {}


def _build_bass():
    import concourse.bass as bass
    import concourse.mybir as mybir
    import concourse.tile as tile
    from concourse.tile_rust import add_dep_helper

    f32 = mybir.dt.float32
    bf16 = mybir.dt.bfloat16
    AF = mybir.ActivationFunctionType
    ALU = mybir.AluOpType
    AX = mybir.AxisListType

    nc = bass.Bass("TRN2")
    ei = nc.dram_tensor("emb_i", [B, D], f32, kind="ExternalInput")
    ej = nc.dram_tensor("emb_j", [B, D], f32, kind="ExternalInput")
    out = nc.dram_tensor("partial", [P, 1], f32, kind="ExternalOutput")

    ei_t = ei.rearrange("(t p) d -> p t d", p=P)   # [128, 32, 128] view of DRAM
    ej_t = ej.rearrange("(t p) d -> p t d", p=P)

    NCHUNK = 4          # load/norm chunks per matrix
    TPC = NT // NCHUNK  # 8 tiles per chunk

    with tile.TileContext(nc) as tc:
        with (
            tc.tile_pool(name="persist", bufs=1) as persist,
            tc.tile_pool(name="scratch", bufs=8) as scratch,
            tc.tile_pool(name="expbuf", bufs=16) as expbuf,
            tc.tile_pool(name="mmpsum", bufs=2, space="PSUM") as mmpsum,
        ):
            zb = persist.tile([P, 1], f32, tag="zb")
            nc.vector.memset(zb, 0.0)

            # persistent tiles
            emb = {}
            z = {}
            zT = {}
            inv = {}
            n2 = {}
            for name in ("i", "j"):
                emb[name] = persist.tile([P, NT, D], bf16, name=f"emb_{name}", tag=f"emb_{name}")
                z[name] = persist.tile([P, NT, D], bf16, name=f"z_{name}", tag=f"z_{name}")
                zT[name] = persist.tile([P, NT, D], bf16, name=f"zT_{name}", tag=f"zT_{name}")
                inv[name] = persist.tile([P, NT], f32, name=f"inv_{name}", tag=f"inv_{name}")
                n2[name] = persist.tile([P, NT], f32, name=f"n2_{name}", tag=f"n2_{name}")

            # loads (SWDGE cast fp32->bf16, one queue => FIFO) + norm stats
            for c in range(NCHUNK):
                for name, dram in (("i", ei_t), ("j", ej_t)):
                    ts = slice(c * TPC, (c + 1) * TPC)
                    nc.gpsimd.dma_start(out=emb[name][:, ts, :], in_=dram[:, ts, :])
                    sq = scratch.tile([P, TPC, D], bf16, tag="sq")
                    nc.vector.tensor_mul(sq, emb[name][:, ts, :], emb[name][:, ts, :])
                    nc.vector.tensor_reduce(
                        out=n2[name][:, ts], in_=sq, axis=AX.X, op=ALU.add
                    )

            # dummy xbar transpose reading the last-loaded chunk: carries the
            # copy->transpose transition wait (1 wait) so the real transposes
            # only need their data-dependency wait.
            dummy_out = persist.tile([P, 16], bf16, tag="dummy_out")
            dummy_inst = nc.sync.dma_start_transpose(
                out=dummy_out, in_=emb["j"][0:16, NT - 1, :]
            )

            for name in ("i", "j"):
                # inv_norm = exp(-0.5 * ln(n2))  -> [128, 32] f32
                lg = scratch.tile([P, NT], f32, tag="lg")
                nc.scalar.activation(lg, n2[name], AF.Ln, bias=zb)
                nc.scalar.activation(inv[name], lg, AF.Exp, bias=zb, scale=-0.5)
                # z = emb * inv_norm (per row)
                for t in range(NT):
                    nc.vector.tensor_scalar_mul(
                        z[name][:, t, :], emb[name][:, t, :], inv[name][:, t : t + 1]
                    )
                # transpose via DMA xbar, in chunks to pipeline. Only 8 HWDGE
                # sem lanes exist and lane reuse adds a second wait (over the
                # XPOSE 1-wait limit), so cap total HWDGE ops at 8: the dummy
                # plus 4 chunks for i and 3 for j.
                bounds = [(0, 8), (8, 16), (16, 24), (24, 32)] if name == "i" else [
                    (0, 11), (11, 22), (22, 32)]
                for lo, hi in bounds:
                    ts = slice(lo, hi)
                    ti = nc.sync.dma_start_transpose(
                        out=zT[name][:, ts, :], in_=z[name][:, ts, :]
                    )
                    add_dep_helper(ti.ins, dummy_inst.ins, False, "xpose after dummy")

            # main similarity + exp + row-sum accumulation
            # branch a: rows from z_i (tiles 0..3), cols = all z_j
            # branch b: rows from z_j, cols = all z_i
            NH = 2               # column halves of 2048
            CW = B // NH         # 2048
            QW = 512             # matmul moving free dim
            NQ = CW // QW        # 4 quads per half
            rsp = {}
            for bname, rows, cols in (("a", "i", "j"), ("b", "j", "i")):
                rsp[bname] = persist.tile([P, NH, MT], f32, name=f"rsp_{bname}", tag=f"rsp_{bname}")
                for m in range(MT):
                    for h in range(NH):
                        ps = mmpsum.tile([P, CW], f32, tag="ps")
                        for q in range(NQ):
                            col0 = h * CW + q * QW
                            nc.tensor.matmul(
                                ps[:, q * QW : (q + 1) * QW],
                                zT[rows][:, m, :],
                                zT[cols].rearrange("p t d -> p (t d)")[
                                    :, col0 : col0 + QW
                                ],
                                start=True,
                                stop=True,
                            )
                        ebuf = expbuf.tile([P, CW], bf16, tag="ebuf")
                        nc.scalar.activation(
                            ebuf,
                            ps,
                            AF.Exp,
                            bias=zb,
                            scale=INV_T,
                            accum_out=rsp[bname][:, h, m : m + 1],
                        )

            # rowsums = sum of the two column-half accumulators -> [128, MT]
            rs = {}
            for bname in ("a", "b"):
                rs[bname] = persist.tile([P, MT], f32, name=f"rs_{bname}", tag=f"rs_{bname}")
                nc.vector.tensor_add(rs[bname], rsp[bname][:, 0, :], rsp[bname][:, 1, :])

            # diag (positives): sum_d z_i[r,d]*z_j[r,d] for r in core's rows
            dtmp = persist.tile([P, MT, D], bf16, tag="dtmp")
            nc.vector.tensor_mul(dtmp, z["i"][:, 0:MT, :], z["j"][:, 0:MT, :])
            diag = persist.tile([P, MT], f32, tag="diag")
            nc.vector.tensor_reduce(out=diag, in_=dtmp, axis=AX.X, op=ALU.add)

            # partial[p] = sum_m [ ln(rs_a) + ln(rs_b) - 2*diag/T ]
            la = persist.tile([P, MT], f32, tag="la")
            lb = persist.tile([P, MT], f32, tag="lb")
            nc.scalar.activation(la, rs["a"], AF.Ln, bias=zb)
            nc.scalar.activation(lb, rs["b"], AF.Ln, bias=zb)
            s1 = persist.tile([P, MT], f32, tag="s1")
            nc.vector.tensor_add(s1, la, lb)
            d8 = persist.tile([P, MT], f32, tag="d8")
            nc.vector.tensor_scalar_mul(d8, diag, -2.0 * INV_T)
            s2 = persist.tile([P, MT], f32, tag="s2")
            nc.vector.tensor_add(s2, s1, d8)
            part = persist.tile([P, 1], f32, tag="part")
            nc.vector.tensor_reduce(out=part, in_=s2, axis=AX.X, op=ALU.add)

            nc.gpsimd.dma_start(out=out[:, :], in_=part)

    return nc


def _split_multi_waits(bir: bytes) -> bytes:
    """The walrus build in this container accepts only ONE sync-wait per
    compute/DMA instruction. Tile emits up to three (slot WAR + engine WAW +
    data deps). Rewrite the BIR: move all but one wait onto standalone
    EventSemaphore instructions inserted just before the offender on the same
    engine queue (same semantics: engine blocks until each wait passes)."""
    import json

    d = json.loads(bir)
    n_split = 0
    for fn in d["functions"]:
        for blk in fn["blocks"]:
            new_insts = []
            for ins in blk["instructions"]:
                si = ins.get("sync_info")
                waits = (si or {}).get("on_wait") or []
                if len(waits) > 1:
                    for w in waits[:-1]:
                        ev = {
                            "debug": ins.get("debug", 0),
                            "engine": ins["engine"],
                            "ins": [],
                            "outs": [],
                            "name": f"{ins['name']}_wsplit{n_split}",
                            "opcode": "EventSemaphore",
                            "sync_info": {"on_update": [], "on_wait": [w]},
                        }
                        n_split += 1
                        new_insts.append(ev)
                    si["on_wait"] = [waits[-1]]
                new_insts.append(ins)
            blk["instructions"] = new_insts
    return json.dumps(d).encode()


def kernel(emb_i: np.ndarray, emb_j: np.ndarray) -> np.ndarray:
    from concourse.bass_utils import run_bass_kernel_spmd

    if "nc" not in _cache:
        nc = _build_bass()
        fixed = _split_multi_waits(nc.to_json_bytes())
        nc.to_json_bytes = lambda: fixed
        _cache["nc"] = nc
    nc = _cache["nc"]

    emb_i = np.ascontiguousarray(emb_i, dtype=np.float32)
    emb_j = np.ascontiguousarray(emb_j, dtype=np.float32)
    in_maps = []
    for c in range(NCORES):
        r = c * RPC
        in_maps.append(
            {
                "emb_i": np.ascontiguousarray(np.roll(emb_i, -r, axis=0)),
                "emb_j": np.ascontiguousarray(np.roll(emb_j, -r, axis=0)),
            }
        )

    import os
    trace = bool(os.environ.get("KERNEL_TRACE"))
    res = run_bass_kernel_spmd(
        nc, in_maps, core_ids=list(range(NCORES)), trace=trace
    )
    _cache["last_res"] = res
    total = np.float64(0.0)
    for r in res.results:
        total += np.float64(r["partial"].sum(dtype=np.float64))
    loss = total / (2 * B)
    return np.array(loss, dtype=np.float32)



# revision 8
# speedup vs baseline: 1.1832x; 1.1832x over previous
"""Trainium2 Bass kernel for NT-Xent style contrastive loss (v2).

Math (B=4096, D=128, T=0.25):
  z_i = normalize(emb_i), z_j = normalize(emb_j)   (L2, per row)
  S = z_i @ z_j^T                                   [B, B]
  loss = (1/2B) * sum_r [ -2*S[r,r]/T + ln(rowsum_c exp(S[r,c]/T))
                                      + ln(colsum_r exp(S[r,c]/T)) ]
(branch-b row sums of exp(S^T) == column sums of exp(S), so exp(S) is
computed exactly ONCE — half the ACT and PE work of the two-branch form.)

Sharding: row-parallel. Core k gets its own 512 rows of emb_i plus the full
emb_j rotated so its matching rows sit first. It computes E = exp(S_blk/T)
for its [512, 4096] block, row sums via ACT accum_out, column sums via
mask-matmuls accumulated in one PSUM bank, positives from the diag. Outputs
a per-partition scalar partial [128,1] and a colsum partial [8,512]; host
sums partials, aligns+sums colsums across cores, takes ln, and combines.

Per-core dataflow:
  - SWDGE cast-DMA loads (fp32 DRAM -> bf16 SBUF), 4 chunks of emb_j + own
    emb_i, pipelined with DVE square+reduce row-norm stats
  - ACT: inv_j = Exp(-0.5 Ln(n2)); inv4_i = Exp(-0.5 Ln(n2) + ln 4) = 4/n_i
  - DVE: z_j = emb_j * inv_j (per t-tile); emb_i stays RAW (row norm is
    folded into the exp's per-partition scale)
  - PE pre-warm: zero-effect matmuls (all-zero mask columns) into the
    colsum bank during the norm phase keep HAM at K=8/8 for the main loop
  - DMA xbar transposes (dummy + eT_i + two 16-tile zT_j groups)
  - main loop m=0..3: S chunks A[128,2048]/B[128,1536]/A2[128,512] in PSUM,
    ACT Exp(scale=inv4_i[:,m]) -> ebuf bf16 + accum_out row sums, then
    mask-matmuls accumulate column sums into psC[8,512]
  - tail: ln(rowsums), diag via DVE, partial = sum_m(ln rs - 2*d*inv4),
    DVE copy psC->SBUF, HWDGE DMA out
"""

import numpy as np

B = 4096
D = 128
P = 128
NCORES = 8
RPC = B // NCORES          # 512 rows per core
NT = B // P                # 32 t-tiles of emb_j
MT = RPC // P              # 4 own m-tiles
TEMP = 0.25
INV_T = 1.0 / TEMP         # 4.0
LN4 = float(np.log(4.0))

_cache = {}


def _build_bass():
    import concourse.bass as bass
    import concourse.mybir as mybir
    import concourse.tile as tile
    from concourse.tile_rust import add_dep_helper

    f32 = mybir.dt.float32
    bf16 = mybir.dt.bfloat16
    AF = mybir.ActivationFunctionType
    ALU = mybir.AluOpType
    AX = mybir.AxisListType

    nc = bass.Bass("TRN2")
    ei = nc.dram_tensor("emb_i", [RPC, D], f32, kind="ExternalInput")
    ej = nc.dram_tensor("emb_j", [B, D], f32, kind="ExternalInput")
    out_p = nc.dram_tensor("partial", [P, 1], f32, kind="ExternalOutput")
    out_cs = nc.dram_tensor("colsum", [8, 512], f32, kind="ExternalOutput")

    ei_t = ei.rearrange("(t p) d -> p t d", p=P)   # [128, 4, 128]
    ej_t = ej.rearrange("(t p) d -> p t d", p=P)   # [128, 32, 128]

    NCH = 4
    TPC = NT // NCH            # 8 t-tiles per load chunk

    with tile.TileContext(nc) as tc:
        with (
            tc.tile_pool(name="persist", bufs=1) as persist,
            tc.tile_pool(name="scratch", bufs=4) as scratch,
            tc.tile_pool(name="ebuf", bufs=3) as ebuf,
            tc.tile_pool(name="psA", bufs=1, space="PSUM") as psA,
            tc.tile_pool(name="psB", bufs=1, space="PSUM") as psB,
            tc.tile_pool(name="psC", bufs=1, space="PSUM") as psCp,
        ):
            emb_j = persist.tile([P, NT, D], bf16, tag="emb_j")
            z_j = persist.tile([P, NT, D], bf16, tag="z_j")
            zT_j = persist.tile([P, NT, D], bf16, tag="zT_j")
            emb_i = persist.tile([P, MT, D], bf16, tag="emb_i")
            eT_i = persist.tile([P, MT, D], bf16, tag="eT_i")
            n2j = persist.tile([P, NT], f32, tag="n2j")
            invj = persist.tile([P, NT], f32, tag="invj")
            n2i = persist.tile([P, MT], f32, tag="n2i")
            inv4i = persist.tile([P, MT], f32, tag="inv4i")
            mask = persist.tile([P, 16], bf16, tag="mask")
            rsp = persist.tile([P, MT, 3], f32, tag="rsp")
            dummy_out = persist.tile([P, 16], bf16, tag="dummy_out")

            # colsum accumulator: one PSUM bank; partitions 0..7 hold the 8
            # 512-wide column chunks of this core's (rotated) column space
            psC_full = psCp.tile([P, 512], f32, tag="psC")
            psC = psC_full[0:8, :]

            nc.vector.memset(mask, 0.0)
            nc.vector.memset(mask[:, 8:9], 1.0)
            zb = persist.tile([P, 1], f32, tag="zb")
            nc.vector.memset(zb, 0.0)
            b_ln4 = persist.tile([P, 1], f32, tag="b_ln4")
            nc.vector.memset(b_ln4, LN4)

            # ---- loads + row-norm stats (pipelined in 2 groups) ----
            nc.gpsimd.dma_start(out=emb_j[:, 0:TPC, :], in_=ej_t[:, 0:TPC, :])
            nc.gpsimd.dma_start(out=emb_i, in_=ei_t)
            nc.gpsimd.dma_start(
                out=emb_j[:, TPC : 2 * TPC, :], in_=ej_t[:, TPC : 2 * TPC, :]
            )
            nc.gpsimd.dma_start(
                out=emb_j[:, 2 * TPC : 3 * TPC, :], in_=ej_t[:, 2 * TPC : 3 * TPC, :]
            )
            nc.gpsimd.dma_start(
                out=emb_j[:, 3 * TPC : 4 * TPC, :], in_=ej_t[:, 3 * TPC : 4 * TPC, :]
            )

            def jstats(c):
                ts = slice(c * TPC, (c + 1) * TPC)
                sq = scratch.tile([P, TPC, D], bf16, tag="sq")
                nc.vector.tensor_mul(sq, emb_j[:, ts, :], emb_j[:, ts, :])
                nc.vector.tensor_reduce(
                    out=n2j[:, ts], in_=sq, axis=AX.X, op=ALU.add
                )

            jstats(0)
            sqi = scratch.tile([P, MT, D], bf16, tag="sqi")
            nc.vector.tensor_mul(sqi, emb_i, emb_i)
            nc.vector.tensor_reduce(out=n2i, in_=sqi, axis=AX.X, op=ALU.add)
            jstats(1)

            # inv_j for first half; inv4_i = 4/n_i (bias=ln4 inside exp)
            lg = scratch.tile([P, NT], f32, tag="lg")
            lgi = scratch.tile([P, MT], f32, tag="lgi")
            nc.scalar.activation(lg[:, 0:16], n2j[:, 0:16], AF.Ln, bias=zb)
            nc.scalar.activation(invj[:, 0:16], lg[:, 0:16], AF.Exp, scale=-0.5, bias=zb)
            nc.scalar.activation(lgi, n2i, AF.Ln, bias=zb)
            nc.scalar.activation(inv4i, lgi, AF.Exp, scale=-0.5, bias=b_ln4)

            # z_j scale, first half
            for t in range(16):
                nc.vector.tensor_scalar_mul(
                    z_j[:, t, :], emb_j[:, t, :], invj[:, t : t + 1]
                )

            # PE pre-warm: zero-effect matmuls into psC (mask cols 0:8 are
            # all zeros). First clears the bank via start=True.
            ej_flat = emb_j.rearrange("p t d -> p (t d)")
            for w in range(12):
                nc.tensor.matmul(
                    psC,
                    mask[:, 0:8],
                    ej_flat[:, (w % 8) * 512 : (w % 8) * 512 + 512],
                    start=(w == 0),
                    stop=False,
                )

            # second-half stats + inv + scale
            jstats(2)
            jstats(3)
            nc.scalar.activation(lg[:, 16:32], n2j[:, 16:32], AF.Ln, bias=zb)
            nc.scalar.activation(invj[:, 16:32], lg[:, 16:32], AF.Exp, scale=-0.5, bias=zb)
            for t in range(16, 32):
                nc.vector.tensor_scalar_mul(
                    z_j[:, t, :], emb_j[:, t, :], invj[:, t : t + 1]
                )

            # diag: d[p,m] = sum_d emb_i * z_j_own  (z_j rows 0:4 = own rows)
            dd = scratch.tile([P, MT, D], bf16, tag="dd")
            nc.vector.tensor_mul(dd, emb_i, z_j[:, 0:MT, :])
            dvec = persist.tile([P, MT], f32, tag="dvec")
            nc.vector.tensor_reduce(out=dvec, in_=dd, axis=AX.X, op=ALU.add)

            # ---- transposes (xbar): dummy absorbs mode-switch wait ----
            dummy_inst = nc.sync.dma_start_transpose(
                out=dummy_out, in_=emb_j[0:16, 0, :]
            )
            ti = nc.sync.dma_start_transpose(out=eT_i, in_=emb_i)
            add_dep_helper(ti.ins, dummy_inst.ins, False, "xpose after dummy")
            for lo, hi in ((0, 16), (16, 32)):
                tj = nc.sync.dma_start_transpose(
                    out=zT_j[:, lo:hi, :], in_=z_j[:, lo:hi, :]
                )
                add_dep_helper(tj.ins, dummy_inst.ins, False, "xpose after dummy")

            # ---- main loop ----
            zT_flat = zT_j.rearrange("p t d -> p (t d)")
            QW = 512
            # (pool, width, col0, h0): h = global 512-col chunk index
            chunks = (
                (psA, 2048, 0, 0),
                (psB, 1536, 2048, 4),
                (psA, 512, 3584, 7),
            )
            for m in range(MT):
                for ci, (pool, w, col0, h0) in enumerate(chunks):
                    ps = pool.tile([P, 2048 if pool is psA else 1536], f32, tag="ps")
                    nq = w // QW
                    for q in range(nq):
                        nc.tensor.matmul(
                            ps[:, q * QW : (q + 1) * QW],
                            eT_i[:, m, :],
                            zT_flat[:, col0 + q * QW : col0 + (q + 1) * QW],
                            start=True,
                            stop=True,
                        )
                    eb = ebuf.tile([P, 2048], bf16, tag="eb")
                    nc.scalar.activation(
                        eb[:, 0:w],
                        ps[:, 0:w],
                        AF.Exp,
                        bias=zb,
                        scale=inv4i[:, m : m + 1],
                        accum_out=rsp[:, m, ci : ci + 1],
                    )
                    last = m == MT - 1 and ci == 2
                    for q in range(nq):
                        h = h0 + q
                        nc.tensor.matmul(
                            psC,
                            mask[:, 8 - h : 16 - h],
                            eb[:, q * QW : (q + 1) * QW],
                            start=False,
                            stop=last and q == nq - 1,
                        )

            # ---- tail ----
            rs = persist.tile([P, MT], f32, tag="rs")
            nc.vector.tensor_add(rs, rsp[:, :, 0], rsp[:, :, 1])
            nc.vector.tensor_add(rs, rs, rsp[:, :, 2])
            lnrs = persist.tile([P, MT], f32, tag="lnrs")
            nc.scalar.activation(lnrs, rs, AF.Ln, bias=zb)
            dsc = persist.tile([P, MT], f32, tag="dsc")
            nc.vector.tensor_mul(dsc, dvec, inv4i)
            s2 = persist.tile([P, MT], f32, tag="s2")
            nc.vector.tensor_scalar_mul(s2, dsc, -2.0)
            nc.vector.tensor_add(s2, s2, lnrs)
            part = persist.tile([P, 1], f32, tag="part")
            nc.vector.tensor_reduce(out=part, in_=s2, axis=AX.X, op=ALU.add)

            cs_sb = persist.tile([8, 512], f32, tag="cs_sb")
            nc.vector.tensor_copy(cs_sb, psC)

            nc.sync.dma_start(out=out_p[:, :], in_=part)
            nc.sync.dma_start(out=out_cs[:, :], in_=cs_sb)

    return nc


def _split_multi_waits(bir: bytes) -> bytes:
    """The walrus build in this container accepts only ONE sync-wait per
    compute/DMA instruction. Tile emits up to three. Move all but one wait
    onto standalone EventSemaphore instructions inserted just before the
    offender on the same engine queue."""
    import json

    d = json.loads(bir)
    n_split = 0
    for fn in d["functions"]:
        for blk in fn["blocks"]:
            new_insts = []
            for ins in blk["instructions"]:
                si = ins.get("sync_info")
                waits = (si or {}).get("on_wait") or []
                if len(waits) > 1:
                    for w in waits[:-1]:
                        ev = {
                            "debug": ins.get("debug", 0),
                            "engine": ins["engine"],
                            "ins": [],
                            "outs": [],
                            "name": f"{ins['name']}_wsplit{n_split}",
                            "opcode": "EventSemaphore",
                            "sync_info": {"on_update": [], "on_wait": [w]},
                        }
                        n_split += 1
                        new_insts.append(ev)
                    si["on_wait"] = [waits[-1]]
                new_insts.append(ins)
            blk["instructions"] = new_insts
    return json.dumps(d).encode()


def kernel(emb_i: np.ndarray, emb_j: np.ndarray) -> np.ndarray:
    from concourse.bass_utils import run_bass_kernel_spmd

    if "nc" not in _cache:
        nc = _build_bass()
        fixed = _split_multi_waits(nc.to_json_bytes())
        nc.to_json_bytes = lambda: fixed
        _cache["nc"] = nc
    nc = _cache["nc"]

    emb_i = np.ascontiguousarray(emb_i, dtype=np.float32)
    emb_j = np.ascontiguousarray(emb_j, dtype=np.float32)
    in_maps = []
    for c in range(NCORES):
        r = c * RPC
        in_maps.append(
            {
                "emb_i": np.ascontiguousarray(emb_i[r : r + RPC]),
                "emb_j": np.ascontiguousarray(np.roll(emb_j, -r, axis=0)),
            }
        )

    import os

    trace = bool(os.environ.get("KERNEL_TRACE"))
    res = run_bass_kernel_spmd(
        nc, in_maps, core_ids=list(range(NCORES)), trace=trace
    )
    _cache["last_res"] = res

    # host combine: sum partials; align + sum colsums, ln, sum
    total = np.float64(0.0)
    cs_total = np.zeros(B, dtype=np.float64)
    for c, r in enumerate(res.results):
        total += np.float64(r["partial"].sum(dtype=np.float64))
        cs = r["colsum"].reshape(B).astype(np.float64)
        cs_total += np.roll(cs, c * RPC)
    total += np.log(cs_total).sum()
    loss = total / (2 * B)
    return np.array(loss, dtype=np.float32)


# revision 11
# speedup vs baseline: 1.5784x; 1.3340x over previous
"""Trainium2 Bass kernel for NT-Xent style contrastive loss (v3, flipped).

Math (B=4096, D=128, T=0.25), with z = row-normalized emb:
  S = z_i @ z_j^T   [B, B]
  loss = (1/2B) * sum_r [ -2*S[r,r]/T + ln(sum_c exp(S[r,c]/T))
                                      + ln(sum_c exp(S[c,r]/T)) ]
exp(S) is computed exactly once; row sums and column sums of it feed the
two ln branches.

Sharding: 2D. Core (rb, ch) with rb = core//2, ch = core%2 owns the
[1024 x 2048] block rows 1024*rb.., cols 2048*ch... All inputs are plain
row slices of the full arrays (no host rotation).

Orientation: the kernel computes S^T chunks [128 cols, 1024 rows]:
  stationary = RAW eT_j c-tile [d, 128 cols], moving = normalized zT_i
  [d, 1024 rows]. Column norms then sit on PSUM partitions, so:
  - exp's per-partition scale folds in 4/n_j (column norm + 1/T) for free
  - ACT accum_out per c-tile = column sums (branch b) directly
  - mask-matmuls summing over partitions = row-sum partials (branch a),
    accumulated across all 16 c-tiles in a single PSUM bank
  - column norms n2_j come from a ones-vector matmul per c-tile of the
    squared transposed block — landing directly in [128 cols, tc] layout
Host sums partials across cores (pairs for row sums, quads for col sums),
takes ln, adds the diag partials, divides by 2B.

Per-core dataflow:
  - SWDGE cast-DMA (fp32->bf16): cj = emb_j col block [128,16,128] in 2
    chunks, ai = own emb_i [128,8,128], oj = own emb_j rows [128,8,128]
  - DVE: row norms for ai (sq+reduce), z_i = ai * inv_i; oj norms for diag
  - xbar transposes: dummy, zT_i, eT_j (2 groups of 8 c-tiles)
  - PE: n2c[tc] = ones-matmul over sqT_j tile -> psn2T [128,16] PSUM;
    ACT ln/exp -> inv4c = 4/n_c
  - PE pre-warm matmuls (zero-mask into the rowsum bank) right before the
    main loop, gated on zT_i, keep HAM at full clock
  - main loop tc=0..15: 2 matmuls [128,512] -> ps [128,1024] (triple
    buffered), ACT Exp(scale=inv4c[:,tc]) -> eb bf16 + accum_out colsum,
    2 mask-matmuls accumulate row-sum partials into psR [2,512]
  - tail: diag partial, DVE copy psR->SBUF, 2 HWDGE output DMAs
"""

import numpy as np

B = 4096
D = 128
P = 128
NCORES = 8
RB = 1024                  # rows per core
CB = 2048                  # cols per core
RT = RB // P               # 8 row t-tiles
CT = CB // P               # 16 col t-tiles
TEMP = 0.25
LN4 = float(np.log(4.0))

_cache = {}


def _build_bass():
    import concourse.bass as bass
    import concourse.mybir as mybir
    import concourse.tile as tile
    from concourse.tile_rust import add_dep_helper

    f32 = mybir.dt.float32
    bf16 = mybir.dt.bfloat16
    AF = mybir.ActivationFunctionType
    ALU = mybir.AluOpType
    AX = mybir.AxisListType

    nc = bass.Bass("TRN2")
    ai_d = nc.dram_tensor("emb_i_blk", [RB, D], f32, kind="ExternalInput")
    cj_d = nc.dram_tensor("emb_j_cols", [CB, D], f32, kind="ExternalInput")
    oj_d = nc.dram_tensor("emb_j_own", [RB, D], f32, kind="ExternalInput")
    out_cs = nc.dram_tensor("colsum", [P, CT + 1], f32, kind="ExternalOutput")
    out_rs = nc.dram_tensor("rowsum", [2, 512], f32, kind="ExternalOutput")

    ai_t = ai_d.rearrange("(t p) d -> p t d", p=P)   # [128, 8, 128]
    cj_t = cj_d.rearrange("(t p) d -> p t d", p=P)   # [128, 16, 128]
    oj_t = oj_d.rearrange("(t p) d -> p t d", p=P)   # [128, 8, 128]

    with tile.TileContext(nc) as tc:
        with (
            tc.tile_pool(name="persist", bufs=1) as persist,
            tc.tile_pool(name="scratch", bufs=4) as scratch,
            tc.tile_pool(name="ebuf", bufs=3) as ebuf,
            tc.tile_pool(name="psmain", bufs=3, space="PSUM") as psmain,
            tc.tile_pool(name="psaux1", bufs=1, space="PSUM") as psaux1,
            tc.tile_pool(name="psaux2", bufs=1, space="PSUM") as psaux2,
        ):
            cj = persist.tile([P, CT, D], bf16, tag="cj")
            eT_j = persist.tile([P, CT, D], bf16, tag="eT_j")
            sqT = persist.tile([P, CT, D], bf16, tag="sqT")
            ai = persist.tile([P, RT, D], bf16, tag="ai")
            z_i = persist.tile([P, RT, D], bf16, tag="z_i")
            zT_i = persist.tile([P, RT, D], bf16, tag="zT_i")
            oj = persist.tile([P, RT, D], bf16, tag="oj")
            mask = persist.tile([P, 16], bf16, tag="mask")
            ones = persist.tile([P, 1], bf16, tag="ones")
            zb = persist.tile([P, 1], f32, tag="zb")
            b_ln4 = persist.tile([P, 1], f32, tag="b_ln4")
            dummy_out = persist.tile([P, 16], bf16, tag="dummy_out")

            n2i = persist.tile([P, RT], f32, tag="n2i")
            invi = persist.tile([P, RT], f32, tag="invi")
            n2o = persist.tile([P, RT], f32, tag="n2o")
            inv4o = persist.tile([P, RT], f32, tag="inv4o")
            inv4c = persist.tile([P, CT], f32, tag="inv4c")
            cs_sb = persist.tile([P, CT + 1], f32, tag="cs_sb")
            rs_sb = persist.tile([2, 512], f32, tag="rs_sb")

            # PSUM: psmain 3 x [128,1024] (6 banks), psR (1), psn2T (1)
            psR_full = psaux1.tile([P, 512], f32, tag="psR")
            psR = psR_full[0:2, :]
            psn2T = psaux2.tile([P, CT], f32, tag="psn2T")

            nc.vector.memset(mask, 0.0)
            nc.vector.memset(mask[:, 8:9], 1.0)
            nc.vector.memset(ones, 1.0)
            nc.vector.memset(zb, 0.0)
            nc.vector.memset(b_ln4, LN4)

            # ---- loads ----
            nc.gpsimd.dma_start(out=ai, in_=ai_t)
            nc.gpsimd.dma_start(out=cj[:, 0:8, :], in_=cj_t[:, 0:8, :])
            nc.gpsimd.dma_start(out=cj[:, 8:16, :], in_=cj_t[:, 8:16, :])
            nc.gpsimd.dma_start(out=oj, in_=oj_t)

            # ---- z_i row norms + scale (DVE + ACT) ----
            sqi = scratch.tile([P, RT, D], bf16, tag="sqi")
            nc.vector.tensor_mul(sqi, ai, ai)
            nc.vector.tensor_reduce(out=n2i, in_=sqi, axis=AX.X, op=ALU.add)
            lgi = scratch.tile([P, RT], f32, tag="lgi")
            nc.scalar.activation(lgi, n2i, AF.Ln, bias=zb)
            nc.scalar.activation(invi, lgi, AF.Exp, scale=-0.5, bias=zb)
            for t in range(RT):
                nc.vector.tensor_scalar_mul(
                    z_i[:, t, :], ai[:, t, :], invi[:, t : t + 1]
                )

            # ---- transposes ----
            dummy_inst = nc.sync.dma_start_transpose(
                out=dummy_out, in_=ai[0:16, 0, :]
            )
            tzi = nc.sync.dma_start_transpose(out=zT_i, in_=z_i)
            add_dep_helper(tzi.ins, dummy_inst.ins, False, "xpose after dummy")
            for lo, hi in ((0, 8), (8, 16)):
                tj = nc.sync.dma_start_transpose(
                    out=eT_j[:, lo:hi, :], in_=cj[:, lo:hi, :]
                )
                add_dep_helper(tj.ins, dummy_inst.ins, False, "xpose after dummy")

            # ---- column norms via PE: n2c[:, tc] = sum_d eT_j[d, tc, :]^2
            for g in (0, 1):
                ts = slice(g * 8, (g + 1) * 8)
                nc.vector.tensor_mul(sqT[:, ts, :], eT_j[:, ts, :], eT_j[:, ts, :])
                for tcc in range(g * 8, (g + 1) * 8):
                    nc.tensor.matmul(
                        psn2T[:, tcc : tcc + 1],
                        sqT[:, tcc, :],
                        ones,
                        start=True,
                        stop=True,
                    )
            lnc = scratch.tile([P, CT], f32, tag="lnc")
            nc.scalar.activation(lnc, psn2T, AF.Ln, bias=zb)
            nc.scalar.activation(inv4c, lnc, AF.Exp, scale=-0.5, bias=b_ln4)

            # ---- diag stats (DVE) ----
            sqo = scratch.tile([P, RT, D], bf16, tag="sqo")
            nc.vector.tensor_mul(sqo, oj, oj)
            nc.vector.tensor_reduce(out=n2o, in_=sqo, axis=AX.X, op=ALU.add)
            lgo = scratch.tile([P, RT], f32, tag="lgo")
            nc.scalar.activation(lgo, n2o, AF.Ln, bias=zb)
            nc.scalar.activation(inv4o, lgo, AF.Exp, scale=-0.5, bias=b_ln4)
            ddt = scratch.tile([P, RT, D], bf16, tag="ddt")
            nc.vector.tensor_mul(ddt, z_i, oj)
            dvec = persist.tile([P, RT], f32, tag="dvec")
            nc.vector.tensor_reduce(out=dvec, in_=ddt, axis=AX.X, op=ALU.add)

            # ---- PE pre-warm: zero-effect matmuls gated on zT_i ----
            zTi_flat = zT_i.rearrange("p t d -> p (t d)")
            for w in range(10):
                nc.tensor.matmul(
                    psR,
                    mask[:, 0:2],
                    zTi_flat[:, (w % 2) * 512 : (w % 2) * 512 + 512],
                    start=(w == 0),
                    stop=False,
                )

            # ---- main loop over 16 c-tiles ----
            for tcc in range(CT):
                ps = psmain.tile([P, 1024], f32, tag="ps")
                for q in range(2):
                    nc.tensor.matmul(
                        ps[:, q * 512 : (q + 1) * 512],
                        eT_j[:, tcc, :],
                        zTi_flat[:, q * 512 : (q + 1) * 512],
                        start=True,
                        stop=True,
                    )
                eb = ebuf.tile([P, 1024], bf16, tag="eb")
                nc.scalar.activation(
                    eb,
                    ps,
                    AF.Exp,
                    bias=zb,
                    scale=inv4c[:, tcc : tcc + 1],
                    accum_out=cs_sb[:, tcc : tcc + 1],
                )
                for q in range(2):
                    nc.tensor.matmul(
                        psR,
                        mask[:, 8 - q : 10 - q],
                        eb[:, q * 512 : (q + 1) * 512],
                        start=False,
                        stop=(tcc == CT - 1 and q == 1),
                    )

            # ---- tail ----
            dsc = scratch.tile([P, RT], f32, tag="dsc")
            nc.vector.tensor_mul(dsc, dvec, inv4o)
            nc.vector.tensor_scalar_mul(dsc, dsc, -2.0)
            nc.vector.tensor_reduce(
                out=cs_sb[:, CT : CT + 1], in_=dsc, axis=AX.X, op=ALU.add
            )
            nc.vector.tensor_copy(rs_sb, psR)

            nc.sync.dma_start(out=out_cs[:, :], in_=cs_sb)
            nc.sync.dma_start(out=out_rs[:, :], in_=rs_sb)

    return nc


def _split_multi_waits(bir: bytes) -> bytes:
    """The walrus build in this container accepts only ONE sync-wait per
    compute/DMA instruction. Tile emits up to three. Move all but one wait
    onto standalone EventSemaphore instructions inserted just before the
    offender on the same engine queue."""
    import json

    d = json.loads(bir)
    n_split = 0
    for fn in d["functions"]:
        for blk in fn["blocks"]:
            new_insts = []
            for ins in blk["instructions"]:
                si = ins.get("sync_info")
                waits = (si or {}).get("on_wait") or []
                if len(waits) > 1:
                    for w in waits[:-1]:
                        ev = {
                            "debug": ins.get("debug", 0),
                            "engine": ins["engine"],
                            "ins": [],
                            "outs": [],
                            "name": f"{ins['name']}_wsplit{n_split}",
                            "opcode": "EventSemaphore",
                            "sync_info": {"on_update": [], "on_wait": [w]},
                        }
                        n_split += 1
                        new_insts.append(ev)
                    si["on_wait"] = [waits[-1]]
                new_insts.append(ins)
            blk["instructions"] = new_insts
    return json.dumps(d).encode()


def kernel(emb_i: np.ndarray, emb_j: np.ndarray) -> np.ndarray:
    from concourse.bass_utils import run_bass_kernel_spmd

    if "nc" not in _cache:
        nc = _build_bass()
        fixed = _split_multi_waits(nc.to_json_bytes())
        nc.to_json_bytes = lambda: fixed
        _cache["nc"] = nc
    nc = _cache["nc"]

    emb_i = np.ascontiguousarray(emb_i, dtype=np.float32)
    emb_j = np.ascontiguousarray(emb_j, dtype=np.float32)
    in_maps = []
    for c in range(NCORES):
        rb, ch = c // 2, c % 2
        in_maps.append(
            {
                "emb_i_blk": emb_i[rb * RB : (rb + 1) * RB],
                "emb_j_cols": emb_j[ch * CB : (ch + 1) * CB],
                "emb_j_own": emb_j[rb * RB : (rb + 1) * RB],
            }
        )

    import os

    trace = bool(os.environ.get("KERNEL_TRACE"))
    res = run_bass_kernel_spmd(
        nc, in_maps, core_ids=list(range(NCORES)), trace=trace
    )
    _cache["last_res"] = res

    # host combine
    dtot = np.float64(0.0)
    cs_total = np.zeros(B, dtype=np.float64)
    rs_total = np.zeros(B, dtype=np.float64)
    for c, r in enumerate(res.results):
        rb, ch = c // 2, c % 2
        cs = r["colsum"]
        # cs[:, tc] covers global col  ch*CB + tc*128 + p
        cs_total[ch * CB : (ch + 1) * CB] += (
            cs[:, :CT].T.reshape(CB).astype(np.float64)
        )
        dtot += np.float64(cs[:, CT].sum(dtype=np.float64))
        rs_total[rb * RB : (rb + 1) * RB] += (
            r["rowsum"].reshape(RB).astype(np.float64)
        )
    total = dtot + np.log(rs_total).sum() + np.log(cs_total).sum()
    loss = total / (2 * B)
    return np.array(loss, dtype=np.float32)


# revision 15
# speedup vs baseline: 1.6002x; 1.0138x over previous
"""Trainium2 Bass kernel for NT-Xent style contrastive loss (v4).

Math (B=4096, D=128, T=0.25), with z = row-normalized emb:
  S = z_i @ z_j^T   [B, B]
  loss = (1/2B) * sum_r [ -2*S[r,r]/T + ln(sum_c exp(S[r,c]/T))
                                      + ln(sum_c exp(S[c,r]/T)) ]
exp(S) is computed exactly once; row sums and column sums of it feed the
two ln branches.

Sharding: 2D. Core (rb, ch), rb = core//2, ch = core%2, owns the
[1024 rows x 2048 cols] block. All inputs are plain row slices (no host
rotation).

Orientation: S^T chunks [128 cols, rows]: stationary = scaled column
tile zcjT [d, 128 c], moving = normalized zT_i [d, 1024 r]. Both norm
factors are pre-applied to the operands (rows: z_i = ai/n_i; cols:
zcj = cj * 4/n_c, absorbing 1/T), so PSUM holds s/T directly and the
exp has a constant scale — ACT chunks span 2 c-tiles [128, 2048].

Reductions:
  - row-sum partials (branch a): Esum[c_p, r] = sum_tc exp-tile, built by
    DVE tensor_tensor_reduce ping-pong; 2 final mask-matmuls reduce the
    128 partitions -> psR [2, 512]
  - col sums (branch b): the same TTR's accum_out gives RUNNING column
    sums; host takes telescoping differences
  - diag (positives): DVE dot of z_i and own emb_j rows
Host sums partials across cores (pairs for row sums, quads for col sums),
takes ln, adds diag partials, divides by 2B.
"""

import numpy as np

B = 4096
D = 128
P = 128
NCORES = 8
RB = 1024                  # rows per core
CB = 2048                  # cols per core
RT = RB // P               # 8 row t-tiles
CT = CB // P               # 16 col t-tiles
TEMP = 0.25
LN4 = float(np.log(4.0))

_cache = {}


def _build_bass():
    import concourse.bass as bass
    import concourse.mybir as mybir
    import concourse.tile as tile
    from concourse.tile_rust import add_dep_helper

    f32 = mybir.dt.float32
    bf16 = mybir.dt.bfloat16
    AF = mybir.ActivationFunctionType
    ALU = mybir.AluOpType
    AX = mybir.AxisListType

    nc = bass.Bass("TRN2")
    ai_d = nc.dram_tensor("emb_i_blk", [RB, D], f32, kind="ExternalInput")
    cj_d = nc.dram_tensor("emb_j_cols", [CB, D], f32, kind="ExternalInput")
    oj_d = nc.dram_tensor("emb_j_own", [RB, D], f32, kind="ExternalInput")
    out_cs = nc.dram_tensor("colsum", [P, CT + 1], f32, kind="ExternalOutput")
    out_rs = nc.dram_tensor("rowsum", [2, 512], f32, kind="ExternalOutput")

    ai_t = ai_d.rearrange("(t p) d -> p t d", p=P)   # [128, 8, 128]
    cj_t = cj_d.rearrange("(t p) d -> p t d", p=P)   # [128, 16, 128]
    oj_t = oj_d.rearrange("(t p) d -> p t d", p=P)   # [128, 8, 128]

    with tile.TileContext(nc) as tc:
        with (
            tc.tile_pool(name="persist", bufs=1) as persist,
            tc.tile_pool(name="scratch", bufs=4) as scratch,
            tc.tile_pool(name="ebuf", bufs=2) as ebuf,
            tc.tile_pool(name="psmain", bufs=2, space="PSUM") as psmain,
        ):
            cj = persist.tile([P, CT, D], bf16, tag="cj")
            zcj = persist.tile([P, CT, D], bf16, tag="zcj")
            zcjT = persist.tile([P, CT, D], bf16, tag="zcjT")
            ai = persist.tile([P, RT, D], bf16, tag="ai")
            z_i = persist.tile([P, RT, D], bf16, tag="z_i")
            zT_i = persist.tile([P, RT, D], bf16, tag="zT_i")
            oj = persist.tile([P, RT, D], bf16, tag="oj")
            mask = persist.tile([P, 16], bf16, tag="mask")
            zb = persist.tile([P, 1], f32, tag="zb")
            b_ln4 = persist.tile([P, 1], f32, tag="b_ln4")
            dummy_out = persist.tile([P, 16], bf16, tag="dummy_out")

            n2i = persist.tile([P, RT], f32, tag="n2i")
            invi = persist.tile([P, RT], f32, tag="invi")
            n2o = persist.tile([P, RT], f32, tag="n2o")
            inv4o = persist.tile([P, RT], f32, tag="inv4o")
            n2c = persist.tile([P, CT], f32, tag="n2c")
            inv4c = persist.tile([P, CT], f32, tag="inv4c")
            cs_sb = persist.tile([P, CT + 1], f32, tag="cs_sb")
            rs_sb = persist.tile([2, 512], f32, tag="rs_sb")
            ezero = persist.tile([P, RB], bf16, tag="ezero")
            esum = [
                persist.tile([P, RB], bf16, name="esum0", tag="esum0"),
                persist.tile([P, RB], bf16, name="esum1", tag="esum1"),
            ]

            nc.vector.memset(mask, 0.0)
            nc.vector.memset(mask[:, 8:9], 1.0)
            nc.vector.memset(zb, 0.0)
            nc.vector.memset(b_ln4, LN4)
            nc.vector.memset(ezero, 0.0)

            # ---- loads (SWDGE cast fp32->bf16, one FIFO queue) ----
            nc.gpsimd.dma_start(out=ai, in_=ai_t)
            nc.gpsimd.dma_start(out=cj[:, 0:8, :], in_=cj_t[:, 0:8, :])
            nc.gpsimd.dma_start(out=cj[:, 8:16, :], in_=cj_t[:, 8:16, :])
            nc.gpsimd.dma_start(out=oj, in_=oj_t)

            # ---- z_i row norms + scale ----
            sqi = scratch.tile([P, RT, D], bf16, tag="sqi")
            nc.vector.tensor_mul(sqi, ai, ai)
            nc.vector.tensor_reduce(out=n2i, in_=sqi, axis=AX.X, op=ALU.add)
            lgi = scratch.tile([P, RT], f32, tag="lgi")
            nc.scalar.activation(lgi, n2i, AF.Ln, bias=zb)
            nc.scalar.activation(invi, lgi, AF.Exp, scale=-0.5, bias=zb)
            for t in range(RT):
                nc.vector.tensor_scalar_mul(
                    z_i[:, t, :], ai[:, t, :], invi[:, t : t + 1]
                )

            # ---- column norms + scale (zcj = cj * 4/n_c), per 8-tile group
            def jgroup(g):
                ts = slice(g * 8, (g + 1) * 8)
                sq = scratch.tile([P, 8, D], bf16, tag="sqj")
                nc.vector.tensor_mul(sq, cj[:, ts, :], cj[:, ts, :])
                nc.vector.tensor_reduce(out=n2c[:, ts], in_=sq, axis=AX.X, op=ALU.add)
                lgc = scratch.tile([P, 8], f32, tag="lgc")
                nc.scalar.activation(lgc, n2c[:, ts], AF.Ln, bias=zb)
                nc.scalar.activation(
                    inv4c[:, ts], lgc, AF.Exp, scale=-0.5, bias=b_ln4
                )
                for t in range(g * 8, (g + 1) * 8):
                    nc.vector.tensor_scalar_mul(
                        zcj[:, t, :], cj[:, t, :], inv4c[:, t : t + 1]
                    )

            jgroup(0)
            jgroup(1)

            # ---- transposes ----
            dummy_inst = nc.sync.dma_start_transpose(
                out=dummy_out, in_=ai[0:16, 0, :]
            )
            tzi = nc.sync.dma_start_transpose(out=zT_i, in_=z_i)
            add_dep_helper(tzi.ins, dummy_inst.ins, False, "xpose after dummy")
            for lo, hi in ((0, 8), (8, 16)):
                tj = nc.sync.dma_start_transpose(
                    out=zcjT[:, lo:hi, :], in_=zcj[:, lo:hi, :]
                )
                add_dep_helper(tj.ins, dummy_inst.ins, False, "xpose after dummy")

            # ---- diag stats ----
            sqo = scratch.tile([P, RT, D], bf16, tag="sqo")
            nc.vector.tensor_mul(sqo, oj, oj)
            nc.vector.tensor_reduce(out=n2o, in_=sqo, axis=AX.X, op=ALU.add)
            lgo = scratch.tile([P, RT], f32, tag="lgo")
            nc.scalar.activation(lgo, n2o, AF.Ln, bias=zb)
            nc.scalar.activation(inv4o, lgo, AF.Exp, scale=-0.5, bias=b_ln4)
            ddt = scratch.tile([P, RT, D], bf16, tag="ddt")
            nc.vector.tensor_mul(ddt, z_i, oj)
            dvec = persist.tile([P, RT], f32, tag="dvec")
            nc.vector.tensor_reduce(out=dvec, in_=ddt, axis=AX.X, op=ALU.add)

            # ---- PE pre-warm (throwaway, gated on zT_i) ----
            zTi_flat = zT_i.rearrange("p t d -> p (t d)")
            ps_warm = psmain.tile([P, 2048], f32, tag="ps")
            for w in range(10):
                nc.tensor.matmul(
                    ps_warm[0:2, (w % 4) * 512 : (w % 4) * 512 + 512],
                    mask[:, 0:2],
                    zTi_flat[:, (w % 2) * 512 : (w % 2) * 512 + 512],
                    start=True,
                    stop=True,
                )

            # ---- main loop: 8 chunks of 2 c-tiles ----
            for k in range(8):
                ps = psmain.tile([P, 2048], f32, tag="ps")
                for sub in range(2):
                    tcc = 2 * k + sub
                    for q in range(2):
                        nc.tensor.matmul(
                            ps[:, sub * 1024 + q * 512 : sub * 1024 + (q + 1) * 512],
                            zcjT[:, tcc, :],
                            zTi_flat[:, q * 512 : (q + 1) * 512],
                            start=True,
                            stop=True,
                        )
                eb = ebuf.tile([P, 2048], bf16, tag="eb")
                nc.scalar.activation(eb, ps, AF.Exp, bias=zb)
                for sub in range(2):
                    tcc = 2 * k + sub
                    prev = ezero if tcc == 0 else esum[(tcc - 1) % 2]
                    nc.vector.scalar_tensor_tensor(
                        out=esum[tcc % 2],
                        in0=eb[:, sub * RB : (sub + 1) * RB],
                        scalar=1.0,
                        in1=prev,
                        op0=ALU.mult,
                        op1=ALU.add,
                        accum_out=cs_sb[:, tcc : tcc + 1],
                    )

            # ---- tail: rowsum partials via 2 mask-matmuls on final Esum
            e_last = esum[(CT - 1) % 2]
            psR_full = psmain.tile([P, 2048], f32, tag="ps")
            psR = psR_full[0:2, 0:512]
            for q in range(2):
                nc.tensor.matmul(
                    psR,
                    mask[:, 8 - q : 10 - q],
                    e_last[:, q * 512 : (q + 1) * 512],
                    start=(q == 0),
                    stop=(q == 1),
                )

            dsc = scratch.tile([P, RT], f32, tag="dsc")
            nc.vector.tensor_mul(dsc, dvec, inv4o)
            nc.vector.tensor_scalar_mul(dsc, dsc, -2.0)
            nc.vector.tensor_reduce(
                out=cs_sb[:, CT : CT + 1], in_=dsc, axis=AX.X, op=ALU.add
            )
            nc.vector.tensor_copy(rs_sb, psR)

            nc.sync.dma_start(out=out_cs[:, :], in_=cs_sb)
            nc.sync.dma_start(out=out_rs[:, :], in_=rs_sb)

    return nc


def _split_multi_waits(bir: bytes) -> bytes:
    """The walrus build in this container accepts only ONE sync-wait per
    compute/DMA instruction. Tile emits up to three. Move all but one wait
    onto standalone EventSemaphore instructions inserted just before the
    offender on the same engine queue."""
    import json

    d = json.loads(bir)
    n_split = 0
    for fn in d["functions"]:
        for blk in fn["blocks"]:
            new_insts = []
            for ins in blk["instructions"]:
                si = ins.get("sync_info")
                waits = (si or {}).get("on_wait") or []
                if len(waits) > 1:
                    for w in waits[:-1]:
                        ev = {
                            "debug": ins.get("debug", 0),
                            "engine": ins["engine"],
                            "ins": [],
                            "outs": [],
                            "name": f"{ins['name']}_wsplit{n_split}",
                            "opcode": "EventSemaphore",
                            "sync_info": {"on_update": [], "on_wait": [w]},
                        }
                        n_split += 1
                        new_insts.append(ev)
                    si["on_wait"] = [waits[-1]]
                new_insts.append(ins)
            blk["instructions"] = new_insts
    return json.dumps(d).encode()


def kernel(emb_i: np.ndarray, emb_j: np.ndarray) -> np.ndarray:
    from concourse.bass_utils import run_bass_kernel_spmd

    if "nc" not in _cache:
        nc = _build_bass()
        fixed = _split_multi_waits(nc.to_json_bytes())
        nc.to_json_bytes = lambda: fixed
        _cache["nc"] = nc
    nc = _cache["nc"]

    emb_i = np.ascontiguousarray(emb_i, dtype=np.float32)
    emb_j = np.ascontiguousarray(emb_j, dtype=np.float32)
    in_maps = []
    for c in range(NCORES):
        rb, ch = c // 2, c % 2
        in_maps.append(
            {
                "emb_i_blk": emb_i[rb * RB : (rb + 1) * RB],
                "emb_j_cols": emb_j[ch * CB : (ch + 1) * CB],
                "emb_j_own": emb_j[rb * RB : (rb + 1) * RB],
            }
        )

    import os

    trace = bool(os.environ.get("KERNEL_TRACE"))
    res = run_bass_kernel_spmd(
        nc, in_maps, core_ids=list(range(NCORES)), trace=trace
    )
    _cache["last_res"] = res

    # host combine
    dtot = np.float64(0.0)
    cs_total = np.zeros(B, dtype=np.float64)
    rs_total = np.zeros(B, dtype=np.float64)
    for c, r in enumerate(res.results):
        rb, ch = c // 2, c % 2
        cs = r["colsum"].astype(np.float64)
        # cs[:, tc] are RUNNING column sums; telescoping differences
        run = cs[:, :CT]
        per_tile = np.diff(
            np.concatenate([np.zeros((P, 1)), run], axis=1), axis=1
        )
        # per_tile[p, tc] covers global col  ch*CB + tc*128 + p
        cs_total[ch * CB : (ch + 1) * CB] += per_tile.T.reshape(CB)
        dtot += np.float64(cs[:, CT].sum())
        rs_total[rb * RB : (rb + 1) * RB] += (
            r["rowsum"].reshape(RB).astype(np.float64)
        )
    total = dtot + np.log(rs_total).sum() + np.log(cs_total).sum()
    loss = total / (2 * B)
    return np.array(loss, dtype=np.float32)
